# revision 20
# baseline (speedup 1.0000x reference)
"""AlmostFairKCRPSLoss (alpha=1) on 8 TRN2 NeuronCores.

Math (per pixel, m=16 ensemble members x_i, target y):
  skill  = (1/16) sum_i |x_i - y|
  spread = (1/480) sum_{i,j} |x_i - x_j| = (1/240) sum_{i<j} |x_i - x_j|
  out    = mean_px (skill - spread)

Using |a-b| = 2*max(a,b) - a - b, the sum_i x_i terms cancel between skill
and spread, leaving per pixel:
  skill - spread = (1/8)*sum_i max(x_i,y) - (1/120)*sum_{i<j} max(x_i,x_j) - y

Only SUMS OF PAIRWISE MAXES are needed. Engine split per core:
  - VectorE: all maxes via bf16 tensor_max (2x DVE mode). Spread = offset
    sweeps d=1..15 over the member block (120 pairs); skill = 9 small TTs of
    1-2 members vs a stride-0-broadcast target, used as filler while DMAs
    trickle in.
  - TensorE: reduces the spread max tiles with ones-vector matmuls
    accumulated into one PSUM slice.
  - ScalarE: f32->bf16 casts, skill-max reduction via activation accum_out,
    the exact f32 target sum, and the final PSUM->SBUF copy.
Host applies the 1/8 and 1/120 weights and the global mean.

Sharding: pure data parallel over the flat pixel volume: 663552 px / 8 cores
= 82944 px/core = 128 partitions x 648 free.
"""

import os

import numpy as np

# The axon trace path needs an NTFF hook that is absent in this container;
# make sure a stray BASS_TRACE env var cannot route us onto it.
os.environ.setdefault("BASS_NEVER_TRACE", "1")

import concourse.bass as bass
import concourse.bacc as bacc
import concourse.mybir as mybir
from concourse import tile
from concourse.bass_utils import run_bass_kernel_spmd

P = 128            # SBUF partitions
F = 648            # pixels per partition per core
M = 16             # ensemble size
NCORES = 8
NPIX = P * F       # 82944 pixels per core
NPIX_TOTAL = NPIX * NCORES  # 663552
MMCHUNK = 512      # matmul moving free-dim chunk (one PSUM bank)
NSK = 9            # skill TT groups: (0),(15,1),(14,2),...,(9,7),(8)
NACC = NSK + 2     # + target sum col, + ACT-reduced spread tail col

_f32 = mybir.dt.float32
_bf16 = mybir.dt.bfloat16


def _member_order():
    order = []
    lo, hi = 0, M - 1
    while lo <= hi:
        order.append(lo)
        if hi != lo:
            order.append(hi)
        lo += 1
        hi -= 1
    return order


def _sweep_pieces():
    """(d, p0_block, p1_block) emission list: d=15..9 during arrivals, then
    8..1; sweeps with >8 blocks split so PSUM reduction chases closely and
    the final piece is tiny."""
    pieces = []
    for j in range(1, 8):
        pieces.append((M - j, 0, j))
    for d in range(8, 0, -1):
        nblk = M - d
        if nblk <= 8:
            pieces.append((d, 0, nblk))
        elif d > 1:
            pieces.append((d, 0, 8))
            pieces.append((d, 8, nblk))
        else:
            pieces.append((1, 0, 8))
            pieces.append((1, 8, 14))
            pieces.append((1, 14, 15))
    return pieces


def build_graph(loop_k=None):
    nc = bacc.Bacc(
        "TRN2", target_bir_lowering=False, debug=False, num_devices=NCORES
    )
    pred_d = nc.dram_tensor("pred", [M, NPIX], _f32, kind="ExternalInput")
    tgt_d = nc.dram_tensor("target", [1, NPIX], _f32, kind="ExternalInput")
    outp_d = nc.dram_tensor("outp", [1, MMCHUNK], _f32, kind="ExternalOutput")
    outa_d = nc.dram_tensor("outa", [P, NACC], _f32, kind="ExternalOutput")

    pred_ap = pred_d.ap().rearrange("m (p f) -> m p f", p=P)
    tgt_ap = tgt_d.ap().rearrange("o (p f) -> o p f", p=P)
    order = _member_order()
    pieces = _sweep_pieces()

    sp_chunks = []   # (d, p0, c0, c1) 512-col matmul chunks, emission order
    for (d, b0, b1) in pieces:
        if (d, b0, b1) == (1, 14, 15):
            continue   # reduced on ScalarE instead
        c = b0 * F
        while c < b1 * F:
            e = min(c + MMCHUNK, b1 * F)
            sp_chunks.append((d, b0, c, e))
            c = e

    with tile.TileContext(nc) as tc:
        with (
            tc.tile_pool(name="main", bufs=1) as pool,
            tc.tile_pool(name="mx", bufs=3) as mxpool,
            tc.tile_pool(name="mxs", bufs=9) as mxspool,
            tc.tile_pool(name="ps", bufs=1, space="PSUM") as pspool,
        ):
            stage = pool.tile([P, (M + 1) * F], _f32)   # slot 16 = target
            mb = pool.tile([P, (M + 1) * F], _bf16)
            ones = pool.tile([P, 1], _bf16)
            acc = pool.tile([P, NACC], _f32)
            outb = pool.tile([1, MMCHUNK], _f32)
            psum_sp = pspool.tile([1, MMCHUNK], _f32)

            nc.vector.memset(ones[:, :], 1.0)

            import contextlib
            loop_ctx = (
                tc.For_i(0, loop_k, 1) if loop_k else contextlib.nullcontext()
            )

            def cast(m):
                nc.scalar.copy(
                    out=mb[:, bass.ts(m, F)], in_=stage[:, bass.ts(m, F)]
                )

            skill_accums = []

            def emit_skill(g, members):
                nb = len(members)
                src = stage if g < 2 else mb   # first groups: f32, no cast dep
                if g < 2:
                    mx = mxspool.tile([P, 2 * F], _f32, tag="mxsf")
                else:
                    mx = mxspool.tile([P, 2 * F], _bf16, tag="mxs")
                if nb == 1:
                    in0 = src[:, bass.ts(members[0], F)].unsqueeze(1)
                else:
                    lo, hi = min(members), max(members)
                    in0 = (
                        src[:, lo * F : (hi + 1) * F]
                        .rearrange("p (m f) -> p m f", f=F)[:, :: (hi - lo), :]
                    )
                in1 = src[:, bass.ts(M, F)].unsqueeze(1).broadcast_to((P, nb, F))
                out3 = mx[:, 0 : nb * F].rearrange("p (m f) -> p m f", f=F)
                nc.vector.tensor_max(out3, in0, in1)
                # skill reduction deferred to ScalarE after all casts
                skill_accums.append((g, nb, mx))

            def emit_sweep_piece(piece):
                d, b0, b1 = piece
                p0, p1 = b0 * F, b1 * F
                mx = mxpool.tile([P, 8 * F], _bf16, tag="mx")
                nc.vector.tensor_max(
                    mx[:, 0 : p1 - p0], mb[:, p0:p1], mb[:, d * F + p0 : d * F + p1]
                )
                if (d, b0, b1) == (1, 14, 15):
                    nc.scalar.activation(
                        out=mx[:, 0 : p1 - p0],
                        in_=mx[:, 0 : p1 - p0],
                        func=mybir.ActivationFunctionType.Identity,
                        accum_out=acc[:, NSK + 1 : NSK + 2],
                    )
                    return
                for (dd, bb, c0, c1) in sp_chunks:
                    if dd != d or bb != b0:
                        continue
                    nc.tensor.matmul(
                        psum_sp[:, 0 : c1 - c0],
                        ones[:, :],
                        mx[:, c0 - p0 : c1 - p0],
                        start=(dd, bb, c0, c1) == sp_chunks[0],
                        stop=(dd, bb, c0, c1) == sp_chunks[-1],
                    )

            # ---- target: DMA, cast, exact f32 sum on ScalarE ----
            loop_ctx.__enter__()
            nc.sync.dma_start(out=stage[:, bass.ts(M, F)], in_=tgt_ap[0])
            cast(M)
            nc.scalar.activation(
                out=stage[:, bass.ts(M, F)],
                in_=stage[:, bass.ts(M, F)],
                func=mybir.ActivationFunctionType.Identity,
                accum_out=acc[:, NSK : NSK + 1],
            )

            # ---- members: DMA + cast + fillers; sweeps when operands landed
            sweep_iter = iter(pieces)
            emitted = 0
            skill_groups = [[0], [15, 1], [14, 2], [13, 3], [12, 4],
                            [11, 5], [10, 6], [9, 7], [8]]
            gnext = 0
            arrived = set()
            for k, m in enumerate(order):
                nc.sync.dma_start(out=stage[:, bass.ts(m, F)], in_=pred_ap[m])
                cast(m)
                arrived.add(m)
                while gnext < NSK and all(
                    x in arrived for x in skill_groups[gnext]
                ):
                    emit_skill(gnext, skill_groups[gnext])
                    gnext += 1
                if k % 2 == 1 and emitted < 7:
                    emit_sweep_piece(next(sweep_iter))
                    emitted += 1
            # exact f32 target sum (dummy out so stage stays read-only)
            tsdump = mxspool.tile([P, 2 * F], _f32, tag="mxsf")
            nc.scalar.activation(
                out=tsdump[:, 0:F],
                in_=stage[:, bass.ts(M, F)],
                func=mybir.ActivationFunctionType.Identity,
                accum_out=acc[:, NSK : NSK + 1],
            )
            for g, nb, mx in skill_accums:
                nc.scalar.activation(
                    out=mx[:, 0 : nb * F],
                    in_=mx[:, 0 : nb * F],
                    func=mybir.ActivationFunctionType.Identity,
                    accum_out=acc[:, g : g + 1],
                )
            for piece in sweep_iter:
                emit_sweep_piece(piece)

            nc.scalar.copy(out=outb[:, :], in_=psum_sp[:, :])
            nc.sync.dma_start(out=outp_d.ap(), in_=outb[:, :])
            nc.sync.dma_start(out=outa_d.ap(), in_=acc[:, :])
            loop_ctx.__exit__(None, None, None)

    nc.compile()
    return nc


_GRAPH = None


def _get_graph():
    global _GRAPH
    if _GRAPH is None:
        _GRAPH = build_graph()
    return _GRAPH


def run(target, pred, **spmd_kwargs):
    """Returns (scalar_result, BassKernelResults)."""
    target = np.ascontiguousarray(target, dtype=np.float32).reshape(1, NPIX_TOTAL)
    pred = np.ascontiguousarray(pred, dtype=np.float32).reshape(M, NPIX_TOTAL)
    in_maps = []
    for r in range(NCORES):
        sl = slice(r * NPIX, (r + 1) * NPIX)
        in_maps.append(
            {
                "pred": np.ascontiguousarray(pred[:, sl]),
                "target": np.ascontiguousarray(target[:, sl]),
            }
        )
    nc = _get_graph()
    res = run_bass_kernel_spmd(nc, in_maps, list(range(NCORES)), **spmd_kwargs)
    total = 0.0
    for r in range(NCORES):
        oa = res.results[r]["outa"].astype(np.float64)
        sp = res.results[r]["outp"].astype(np.float64).sum() + oa[:, NSK + 1].sum()
        sk = oa[:, 0:NSK].sum()
        tg = oa[:, NSK].sum()
        total += sk / 8.0 - sp / 120.0 - tg
    return np.array(total / NPIX_TOTAL, dtype=np.float32), res


def kernel(target, pred):
    value, _ = run(target, pred)
    return value


# revision 22
# speedup vs baseline: 1.0242x; 1.0242x over previous
"""AlmostFairKCRPSLoss (alpha=1) on 8 TRN2 NeuronCores.

Math (per pixel, m=16 ensemble members x_i, target y):
  skill  = (1/16) sum_i |x_i - y|
  spread = (1/480) sum_{i,j} |x_i - x_j| = (1/240) sum_{i<j} |x_i - x_j|
  out    = mean_px (skill - spread)

Using |a-b| = 2*max(a,b) - a - b, the sum_i x_i terms cancel between skill
and spread, leaving per pixel:
  skill - spread = (1/8)*sum_i max(x_i,y) - (1/120)*sum_{i<j} max(x_i,x_j) - y

Only SUMS OF PAIRWISE MAXES are needed. Engine split per core:
  - VectorE: all maxes via bf16 tensor_max (2x DVE mode). Spread = offset
    sweeps d=1..15 over the member block (120 pairs); skill = 9 small TTs of
    1-2 members vs a stride-0-broadcast target, used as filler while DMAs
    trickle in.
  - TensorE: reduces the spread max tiles with ones-vector matmuls
    accumulated into one PSUM slice.
  - ScalarE: f32->bf16 casts, skill-max reduction via activation accum_out,
    the exact f32 target sum, and the final PSUM->SBUF copy.
Host applies the 1/8 and 1/120 weights and the global mean.

Sharding: pure data parallel over the flat pixel volume: 663552 px / 8 cores
= 82944 px/core = 128 partitions x 648 free.
"""

import os

import numpy as np

# The axon trace path needs an NTFF hook that is absent in this container;
# make sure a stray BASS_TRACE env var cannot route us onto it.
os.environ.setdefault("BASS_NEVER_TRACE", "1")

import concourse.bass as bass
import concourse.bacc as bacc
import concourse.mybir as mybir
from concourse import tile
from concourse.bass_utils import run_bass_kernel_spmd

P = 128            # SBUF partitions
XC = 32            # T-path (transposed) pixel columns per partition
F = 648 - XC       # V-path pixels per partition per core
M = 16             # ensemble size
NCORES = 8
NPIX = P * (F + XC)    # 82944 pixels per core
XPIX = P * XC          # 7168 T-path pixels per core
NTCH = XPIX // 512     # T-path 512-px matmul chunks (14)
NPIX_TOTAL = NPIX * NCORES  # 663552
MMCHUNK = 512      # matmul moving free-dim chunk (one PSUM bank)
NSK = 9            # skill TT groups: (0),(15,1),(14,2),...,(9,7),(8)
NACC = NSK + 2 + 2 * NTCH   # + tsumV, sp-tail, then T-path chunk cols
COL_TSV = NSK               # target sum, V-path pixels
COL_SPT = NSK + 1           # ACT-reduced spread tail block
COL_T0 = NSK + 2            # T-path chunk cols: sp-abs at +2c, sk-abs at +2c+1

_f32 = mybir.dt.float32
_bf16 = mybir.dt.bfloat16


def _member_order():
    order = []
    lo, hi = 0, M - 1
    while lo <= hi:
        order.append(lo)
        if hi != lo:
            order.append(hi)
        lo += 1
        hi -= 1
    return order


def _sweep_pieces():
    """(d, p0_block, p1_block) emission list: d=15..9 during arrivals, then
    8..1; sweeps with >8 blocks split so PSUM reduction chases closely and
    the final piece is tiny."""
    pieces = []
    for j in range(1, 8):
        pieces.append((M - j, 0, j))
    for d in range(8, 0, -1):
        nblk = M - d
        if nblk <= 8:
            pieces.append((d, 0, nblk))
        elif d > 1:
            pieces.append((d, 0, 8))
            pieces.append((d, 8, nblk))
        else:
            pieces.append((1, 0, 8))
            pieces.append((1, 8, 14))
            pieces.append((1, 14, 15))
    return pieces


def build_graph(loop_k=None):
    nc = bacc.Bacc(
        "TRN2", target_bir_lowering=False, debug=False, num_devices=NCORES
    )
    pred_d = nc.dram_tensor("pred", [M, NPIX], _f32, kind="ExternalInput")
    tgt_d = nc.dram_tensor("target", [1, NPIX], _f32, kind="ExternalInput")
    w_d = nc.dram_tensor("wmat", [M + 1, 144], _f32, kind="ExternalInput")
    outp_d = nc.dram_tensor("outp", [1, MMCHUNK], _f32, kind="ExternalOutput")
    outa_d = nc.dram_tensor("outa", [P, NACC], _f32, kind="ExternalOutput")

    pred_ap = pred_d.ap()[:, 0 : P * F].rearrange("m (p f) -> m p f", p=P)
    tgt_ap = tgt_d.ap()[:, 0 : P * F].rearrange("o (p f) -> o p f", p=P)
    predT_ap = pred_d.ap()[:, P * F :]            # (16, XPIX)
    tgtT_ap = tgt_d.ap()[:, P * F :]              # (1, XPIX)

    order = _member_order()
    pieces = _sweep_pieces()

    sp_chunks = []   # (d, p0, c0, c1) 512-col matmul chunks, emission order
    for (d, b0, b1) in pieces:
        if (d, b0, b1) == (1, 14, 15):
            continue   # reduced on ScalarE instead
        c = b0 * F
        while c < b1 * F:
            e = min(c + MMCHUNK, b1 * F)
            sp_chunks.append((d, b0, c, e))
            c = e

    with tile.TileContext(nc) as tc:
        with (
            tc.tile_pool(name="main", bufs=1) as pool,
            tc.tile_pool(name="mx", bufs=3) as mxpool,
            tc.tile_pool(name="mxs", bufs=9) as mxspool,
            tc.tile_pool(name="ps", bufs=1, space="PSUM") as pspool,
            tc.tile_pool(name="pst", bufs=3, space="PSUM") as pstpool,
        ):
            stage = pool.tile([P, (M + 1) * F], _f32)   # slot 16 = target
            mb = pool.tile([P, (M + 1) * F], _bf16)
            ones = pool.tile([P, 1], _bf16)
            acc = pool.tile([P, NACC], _f32)
            outb = pool.tile([1, MMCHUNK], _f32)
            psum_sp = pspool.tile([1, MMCHUNK], _f32)
            xT = pool.tile([M + 1, XPIX], _f32)
            xTb = pool.tile([M + 1, XPIX], _bf16)
            wt = pool.tile([M + 1, 144], _f32)
            wtb = pool.tile([M + 1, 144], _bf16)

            nc.vector.memset(ones[:, :], 1.0)
            nc.vector.memset(acc[:, :], 0.0)

            import contextlib
            loop_ctx = (
                tc.For_i(0, loop_k, 1) if loop_k else contextlib.nullcontext()
            )

            vcast_left = [0]

            def cast(m):
                if vcast_left[0] > 0 and m != M:
                    vcast_left[0] -= 1
                    nc.vector.tensor_copy(
                        mb[:, bass.ts(m, F)], stage[:, bass.ts(m, F)]
                    )
                else:
                    nc.scalar.copy(
                        out=mb[:, bass.ts(m, F)], in_=stage[:, bass.ts(m, F)]
                    )

            skill_accums = []

            def emit_skill(g, members):
                nb = len(members)
                src = stage if g < 2 else mb   # first groups: f32, no cast dep
                if g < 2:
                    mx = mxspool.tile([P, 2 * F], _f32, tag="mxsf")
                else:
                    mx = mxspool.tile([P, 2 * F], _bf16, tag="mxs")
                if nb == 1:
                    in0 = src[:, bass.ts(members[0], F)].unsqueeze(1)
                else:
                    lo, hi = min(members), max(members)
                    in0 = (
                        src[:, lo * F : (hi + 1) * F]
                        .rearrange("p (m f) -> p m f", f=F)[:, :: (hi - lo), :]
                    )
                in1 = src[:, bass.ts(M, F)].unsqueeze(1).broadcast_to((P, nb, F))
                out3 = mx[:, 0 : nb * F].rearrange("p (m f) -> p m f", f=F)
                nc.vector.tensor_max(out3, in0, in1)
                # skill reduction deferred to ScalarE after all casts
                skill_accums.append((g, nb, mx))

            def emit_sweep_piece(piece):
                d, b0, b1 = piece
                p0, p1 = b0 * F, b1 * F
                mx = mxpool.tile([P, 8 * F], _bf16, tag="mx")
                nc.vector.tensor_max(
                    mx[:, 0 : p1 - p0], mb[:, p0:p1], mb[:, d * F + p0 : d * F + p1]
                )
                if (d, b0, b1) == (1, 14, 15):
                    nc.scalar.activation(
                        out=mx[:, 0 : p1 - p0],
                        in_=mx[:, 0 : p1 - p0],
                        func=mybir.ActivationFunctionType.Identity,
                        accum_out=acc[:, COL_SPT : COL_SPT + 1],
                    )
                    return
                for (dd, bb, c0, c1) in sp_chunks:
                    if dd != d or bb != b0:
                        continue
                    nc.tensor.matmul(
                        psum_sp[:, 0 : c1 - c0],
                        ones[:, :],
                        mx[:, c0 - p0 : c1 - p0],
                        start=(dd, bb, c0, c1) == sp_chunks[0],
                        stop=(dd, bb, c0, c1) == sp_chunks[-1],
                    )

            # ---- target: DMA, cast, exact f32 sum on ScalarE ----
            loop_ctx.__enter__()
            nc.sync.dma_start(out=wt[:, :], in_=w_d.ap())
            nc.sync.dma_start(out=stage[:, bass.ts(M, F)], in_=tgt_ap[0])
            cast(M)
            nc.scalar.activation(
                out=stage[:, bass.ts(M, F)],
                in_=stage[:, bass.ts(M, F)],
                func=mybir.ActivationFunctionType.Identity,
                accum_out=acc[:, COL_TSV : COL_TSV + 1],
            )

            # ---- members: DMA + cast + fillers; sweeps when operands landed
            sweep_iter = iter(pieces)
            emitted = 0
            skill_groups = [[0], [15, 1], [14, 2], [13, 3], [12, 4],
                            [11, 5], [10, 6], [9, 7], [8]]
            gnext = 0
            arrived = set()
            for k, m in enumerate(order):
                nc.sync.dma_start(out=stage[:, bass.ts(m, F)], in_=pred_ap[m])
                cast(m)
                arrived.add(m)
                while gnext < NSK and all(
                    x in arrived for x in skill_groups[gnext]
                ):
                    emit_skill(gnext, skill_groups[gnext])
                    gnext += 1
                if k % 2 == 1 and emitted < 7:
                    emit_sweep_piece(next(sweep_iter))
                    emitted += 1
            # ---- T-path: late DMAs; PE bf16 pair-diff matmuls + ACT abs
            nc.sync.dma_start(out=xT[0:M, :], in_=predT_ap)
            nc.sync.dma_start(out=xT[M : M + 1, :], in_=tgtT_ap)

            def emit_tchunk(c):
                c0 = c * 512
                psA = pstpool.tile([P, 512], _f32, tag="psA")
                psB = pstpool.tile([M, 512], _f32, tag="psB")
                nc.tensor.matmul(
                    psA[:, :],
                    wtb[:, 0:128],
                    xTb[:, c0 : c0 + 512],
                    start=True,
                    stop=True,
                )
                nc.scalar.activation(
                    out=psA[:, :],
                    in_=psA[:, :],
                    func=mybir.ActivationFunctionType.Abs,
                    accum_out=acc[:, COL_T0 + 2 * c : COL_T0 + 2 * c + 1],
                )
                nc.tensor.matmul(
                    psB[:, :],
                    wtb[:, 128:144],
                    xTb[:, c0 : c0 + 512],
                    start=True,
                    stop=True,
                )
                nc.scalar.activation(
                    out=psB[:, :],
                    in_=psB[:, :],
                    func=mybir.ActivationFunctionType.Abs,
                    accum_out=acc[0:M, COL_T0 + 2 * c + 1 : COL_T0 + 2 * c + 2],
                )

            # exact f32 target sum (dummy out so stage stays read-only)
            tsdump = mxspool.tile([P, 2 * F], _f32, tag="mxsf")
            nc.scalar.activation(
                out=tsdump[:, 0:F],
                in_=stage[:, bass.ts(M, F)],
                func=mybir.ActivationFunctionType.Identity,
                accum_out=acc[:, COL_TSV : COL_TSV + 1],
            )
            for g, nb, mx in skill_accums:
                nc.scalar.activation(
                    out=mx[:, 0 : nb * F],
                    in_=mx[:, 0 : nb * F],
                    func=mybir.ActivationFunctionType.Identity,
                    accum_out=acc[:, g : g + 1],
                )
            nc.scalar.copy(out=wtb[:, :], in_=wt[:, :])
            nc.scalar.copy(out=xTb[:, :], in_=xT[:, :])
            tch = 0
            rem = [p for p in sweep_iter]
            for pi, piece in enumerate(rem):
                emit_sweep_piece(piece)
                want = (NTCH * (pi + 1)) // len(rem)
                while tch < min(want + 1, NTCH):
                    emit_tchunk(tch)
                    tch += 1
            while tch < NTCH:
                emit_tchunk(tch)
                tch += 1

            nc.scalar.copy(out=outb[:, :], in_=psum_sp[:, :])
            nc.sync.dma_start(out=outp_d.ap(), in_=outb[:, :])
            nc.sync.dma_start(out=outa_d.ap(), in_=acc[:, :])
            loop_ctx.__exit__(None, None, None)

    nc.compile()
    return nc


_GRAPH = None


def _get_graph():
    global _GRAPH
    if _GRAPH is None:
        _GRAPH = build_graph()
    return _GRAPH


def _wmat():
    w = np.zeros((M + 1, 144), dtype=np.float32)
    k = 0
    for i in range(M):
        for j in range(i + 1, M):
            w[i, k] = 1.0
            w[j, k] = -1.0
            k += 1
    assert k == 120
    for m in range(M):
        w[m, 128 + m] = 1.0
        w[M, 128 + m] = -1.0
    return w


def run(target, pred, **spmd_kwargs):
    """Returns (scalar_result, BassKernelResults)."""
    target = np.ascontiguousarray(target, dtype=np.float32).reshape(1, NPIX_TOTAL)
    pred = np.ascontiguousarray(pred, dtype=np.float32).reshape(M, NPIX_TOTAL)
    in_maps = []
    for r in range(NCORES):
        sl = slice(r * NPIX, (r + 1) * NPIX)
        in_maps.append(
            {
                "pred": np.ascontiguousarray(pred[:, sl]),
                "target": np.ascontiguousarray(target[:, sl]),
                "wmat": _wmat(),
            }
        )
    nc = _get_graph()
    try:
        res = run_bass_kernel_spmd(nc, in_maps, list(range(NCORES)), **spmd_kwargs)
    except Exception:
        # transient device errors have been observed on this pool; retry once
        res = run_bass_kernel_spmd(nc, in_maps, list(range(NCORES)), **spmd_kwargs)
    total = 0.0
    for r in range(NCORES):
        oa = res.results[r]["outa"].astype(np.float64)
        # V-path pixels (max identity, S terms cancelled):
        sp_v = res.results[r]["outp"].astype(np.float64).sum() + oa[:, COL_SPT].sum()
        sk_v = oa[:, 0:NSK].sum()
        tg_v = oa[:, COL_TSV].sum()
        # T-path pixels (direct |diff| sums):
        spabs_t = sum(oa[:, COL_T0 + 2 * c].sum() for c in range(NTCH))
        skabs_t = sum(oa[0:M, COL_T0 + 2 * c + 1].sum() for c in range(NTCH))
        total += (
            sk_v / 8.0 - sp_v / 120.0 - tg_v
            + skabs_t / 16.0 - spabs_t / 240.0
        )
    return np.array(total / NPIX_TOTAL, dtype=np.float32), res


def kernel(target, pred):
    value, _ = run(target, pred)
    return value
